# revision 46
# baseline (speedup 1.0000x reference)
"""Causal self-attention (T=2048, C=2048, 16 heads) on 8 TRN2 NeuronCores.

Tensor-parallel over heads: core c owns heads 2c, 2c+1; no collectives —
each core computes the PARTIAL final projection over its 2 head sections
and the host sums the 8 fp16 partials in fp32 (the "all-reduce after
c_proj" happens at host-gather time).

Per-core kernel:
 - q,k projected transposed (qT,kT: [d, T]) in two query-half waves; v in
   natural layout [T, d], emitted as filler inside attention (below)
 - scores transposed: ST[s, t] = kT_blk.T @ qT (keys on partitions),
   causally trimmed at 128-column granularity
 - softmax without max-subtraction (scores are O(+-6) for randn inputs):
   E = exp(scale*ST) masked only on the 128-col diagonal-straddling
   block; denominator l via ones-matmuls, both heads sharing one psum
   bank in PE column-groups 0/64 (tile_position); o = (v.T @ E)*bc(1/l)
 - both heads run interleaved per query chunk: two independent exp
   chains + v/c_proj filler matmuls (cost-weighted, deadline-pinned)
   keep the strictly in-order PE queue from head-of-line blocking on
   the scalar engine's exp latency
 - throughput details: host pre-tiles all inputs into exact SBUF layout
   (fully contiguous DMAs), single sync HWDGE ring in consumption order
   (the 2 rings share HBM bandwidth and completion-semaphore lanes),
   dummy matmuls warm the HAM clock gate (1.2->2.4 GHz) during the DMA
   leading edge and bridge the final normalization chain before the
   eagerly-emitted last c_proj chunk.
"""

import numpy as np
import ml_dtypes

import concourse.mybir as mybir
import concourse.tile as tile
from concourse import bacc
from concourse.bass import ds, ts
from concourse.bass_utils import run_bass_kernel_spmd

T = 2048
C = 2048
H = 16
D = 128            # head dim
NC = 8             # cores
HPC = H // NC      # heads per core
DH = HPC * D       # 256: qkv rows per section per core
KB = C // 128      # 16 contraction tiles
TB = T // 128      # 16 t tiles
NQ = 512           # query chunk (psum bank width)
QC = T // NQ       # 4 query chunks
SCALE = float(1.0 / np.sqrt(D))

BF16 = mybir.dt.bfloat16
F16 = mybir.dt.float16
F32 = mybir.dt.float32
EXP = mybir.ActivationFunctionType.Exp

_CACHED = {}


def build():
    nc = bacc.Bacc("TRN2", target_bir_lowering=False, debug=False,
                   num_devices=NC)
    # all inputs are pre-tiled on the host into the exact SBUF layout so
    # every DMA is fully contiguous on both sides (large descriptor
    # runs -> cheap triggers, full HBM bandwidth).
    # xT: [p, slot=(g*4+n), kb_in_g*512 + t'] with kb = 4g+kb_in_g,
    # t = 512n + t'
    xT = nc.dram_tensor("xT", [128, KB * T], BF16, kind="ExternalInput")
    wqkT = nc.dram_tensor("wqkT", [128, KB * 4 * D], BF16,
                          kind="ExternalInput")
    wvT = nc.dram_tensor("wvT", [128, KB * DH], BF16, kind="ExternalInput")
    wpT = nc.dram_tensor("wpT", [128, HPC * C], BF16, kind="ExternalInput")
    maskT = nc.dram_tensor("maskT", [128, 128], BF16, kind="ExternalInput")
    out = nc.dram_tensor("out", [T, C], F16, kind="ExternalOutput")

    with tile.TileContext(nc) as tc:
        with (
            tc.tile_pool(name="const", bufs=1) as const,
            tc.tile_pool(name="work", bufs=4) as work,
            tc.tile_pool(name="psum", bufs=2, space="PSUM") as psum,
        ):
            # ---------------- input loads ----------------
            # xT_sb: [p, slot=(g*4+n), kb_in_g*512 + t']
            xT_sb = const.tile([128, KB, T], BF16)
            wqk_sb = const.tile([128, KB, 4 * D], BF16)
            wv_sb = const.tile([128, KB, DH], BF16)
            wp_sb = const.tile([128, HPC, C], BF16)
            mask_sb = const.tile([128, 128], BF16)
            warm_sb = const.tile([128, NQ], BF16)
            wqk_r = wqkT.ap().rearrange("p (kb m) -> p kb m", m=4 * D)
            xT_r = xT.ap().rearrange("p (s w) -> p s w", w=T)

            def xs(kb, t0, w):
                # x[kb, t0:t0+w] in slot layout (t0 must stay within one
                # 512-column n-chunk)
                g, i = kb // 4, kb % 4
                n, tp = t0 // NQ, t0 % NQ
                return xT_sb[:, 4 * g + n, ds(512 * i + tp, w)]

            # PE warm-up: the HAM clock gate needs ~3.4us of sustained
            # matmul activity to lift the PE from 1.2 to 2.4 GHz; burn it
            # on dummy matmuls while the first input DMAs are in flight
            nc.vector.memset(warm_sb[:, :], 0.03125)
            ps_w = psum.tile([128, NQ], F32, tag="f", bufs=1, name="ps_warm")
            for i in range(8):
                nc.tensor.matmul(ps_w[:, :], warm_sb[:, 0:128], warm_sb[:, :],
                                 start=(i == 0), stop=(i == 7))
            scrap = work.tile([128, NQ], F32, tag="scrap", bufs=1,
                              name="scrap")
            nc.vector.tensor_copy(scrap[:, :], ps_w[:, :])

            # ALL loads go on the single sync ring in exact consumption
            # order: the two HWDGE rings share the ~358 GB/s HBM port AND
            # the 8 global DMA-completion semaphore lanes, so a second
            # ring only creates cross-ring stalls. Wave 1 runs n-major
            # (all m at n=0 first), so slots arrive just in time.
            # ultra-fine leading edge: the first wave matmul needs only
            # wqk[kb0] (131KB) + x slot0 cols 0:512 (131KB); wave 1's
            # first kb-group runs kb-major to match this arrival order
            nc.sync.dma_start(wqk_sb[:, 0, :], wqk_r[:, 0, :])
            nc.sync.dma_start(xT_sb[:, 0, ts(0, NQ)], xT_r[:, 0, ts(0, NQ)])
            nc.sync.dma_start(wqk_sb[:, 1, :], wqk_r[:, 1, :])
            nc.sync.dma_start(xT_sb[:, 0, ts(1, NQ)], xT_r[:, 0, ts(1, NQ)])
            nc.sync.dma_start(wqk_sb[:, ds(2, 2), :].rearrange("p a b -> p (a b)"),
                              wqkT.ap()[:, ds(2 * 4 * D, 2 * 4 * D)])
            nc.sync.dma_start(xT_sb[:, 0, ds(2 * NQ, T - 2 * NQ)],
                              xT_r[:, 0, ds(2 * NQ, T - 2 * NQ)])
            nc.sync.dma_start(xT_sb[:, 1, :], xT_r[:, 1, :])
            for g in range(1, 4):
                nc.sync.dma_start(wqk_sb[:, ts(g, 4), :].rearrange("p a b -> p (a b)"),
                                  wqkT.ap()[:, ds(g * 4 * 4 * D, 4 * 4 * D)])
                nc.sync.dma_start(xT_sb[:, 4 * g, :], xT_r[:, 4 * g, :])
                nc.sync.dma_start(xT_sb[:, 4 * g + 1, :], xT_r[:, 4 * g + 1, :])
            nc.sync.dma_start(mask_sb[:, :], maskT[:, :])
            nc.sync.dma_start(wv_sb[:, :, :].rearrange("p a b -> p (a b)"),
                              wvT.ap())
            for g in range(4):
                for n in range(2, 4):
                    nc.sync.dma_start(xT_sb[:, 4 * g + n, :],
                                      xT_r[:, 4 * g + n, :])
            nc.sync.dma_start(wp_sb[:, :, :].rearrange("p a b -> p (a b)"),
                              wpT.ap())

            ones_col = const.tile([128, 1], BF16)
            nc.vector.memset(ones_col[:, :], 1.0)

            qk_sb = const.tile([128, 4, T], BF16)      # m: qh0 qh1 kh0 kh1
            v_sb = const.tile([128, TB, DH], BF16)     # v[tb] natural layout

            # ------------- q,k projections, wave 1 (n = 0, 1) -------------
            wave = [(m, n) for n in range(2) for m in range(4)]
            wtags = [("mm", 4)] * 4 + [("o", 2)] * 2 + [("l", 1)] + \
                [("f", 1)]
            wave_ps = {}
            for (m, n), (tg, bf) in zip(wave, wtags):
                wave_ps[(m, n)] = psum.tile([128, NQ], F32, tag=tg, bufs=bf,
                                            name=f"ps_qk_{m}_{n}")
            # first kb-group kb-major (matches the fine leading DMAs);
            # later groups (m,n)-major with kb runs
            for n in range(2):
                for kb in range(4):
                    for m in range(4):
                        nc.tensor.matmul(
                            wave_ps[(m, n)][:, :],
                            wqk_sb[:, kb, ts(m, 128)],
                            xs(kb, n * NQ, NQ),
                            start=(kb == 0), stop=False,
                        )
            for kbg in range(1, 4):
                for m, n in wave:
                    for kb in range(4 * kbg, 4 * kbg + 4):
                        nc.tensor.matmul(
                            wave_ps[(m, n)][:, :],
                            wqk_sb[:, kb, ts(m, 128)],
                            xs(kb, n * NQ, NQ),
                            start=False, stop=(kb == KB - 1),
                        )
            for m, n in wave:
                nc.vector.tensor_copy(qk_sb[:, m, ts(n, NQ)],
                                      wave_ps[(m, n)][:, :])

            # ------------- q,k projections, wave 2 (n = 2, 3) -------------
            def emit_wave2(n):
                for m in range(4):
                    ps = psum.tile([128, NQ], F32, tag="mm", bufs=4,
                                   name=f"ps_qk_{m}_{n}")
                    for kb in range(KB):
                        nc.tensor.matmul(
                            ps[:, :],
                            wqk_sb[:, kb, ts(m, 128)],
                            xs(kb, n * NQ, NQ),
                            start=(kb == 0), stop=(kb == KB - 1),
                        )
                    nc.vector.tensor_copy(qk_sb[:, m, ts(n, NQ)], ps[:, :])

            # ---------------- filler emitters ----------------
            def emit_v(tb):
                # v[tb] natural layout; psum shares the "f" tag (cproj
                # fillers and v fillers never overlap within a chunk)
                psv = psum.tile([128, DH], F32, tag="f", bufs=1,
                                name=f"ps_v_{tb}")
                for kb in range(KB):
                    nc.tensor.matmul(
                        psv[:, :],
                        xs(kb, tb * 128, 128),
                        wv_sb[:, kb, :],
                        start=(kb == 0), stop=(kb == KB - 1),
                    )
                nc.vector.tensor_copy(v_sb[:, tb, :], psv[:, :])

            fo_tiles = {}

            def emit_psf(qc, oTs, tb, jn):
                # one c_proj psum group: partial[t, j] for a 128x512 block
                if jn == 0:
                    fo_tiles[tb] = work.tile([128, QC, NQ], F16, tag="fo",
                                             bufs=4, name=f"fo_{qc}_{tb}")
                fo = fo_tiles[tb]
                psf = psum.tile([128, NQ], F32, tag="f", bufs=1,
                                name=f"ps_f_{qc}_{tb}_{jn}")
                for h in range(HPC):
                    nc.tensor.matmul(
                        psf[:, :],
                        oTs[h][:, ts(tb, 128)],
                        wp_sb[:, h, ds(jn * NQ, NQ)],
                        start=(h == 0), stop=(h == HPC - 1),
                    )
                if jn == 1:
                    nc.scalar.copy(fo[:, jn, :], psf[:, :])
                else:
                    nc.vector.tensor_copy(fo[:, jn, :], psf[:, :])
                if jn == QC - 1:
                    nc.sync.dma_start(out[ds(qc * NQ + tb * 128, 128), :],
                                      fo[:, :, :])

            TAIL_TAGS = [("f", 1), ("o", 2), ("mm", 4), ("l", 1)]

            def emit_psf_tail(qc, oTs, tb, jn):
                # tail variant: no attention matmuls left to hide the
                # psum->sbuf copy latency, so rotate psf groups across
                # ALL psum tags (every bank is free here), split each
                # copy across DVE and ACT, and DMA out per-psf
                if jn == 0:
                    fo_tiles[tb] = work.tile([128, QC, NQ], F16, tag="fo",
                                             bufs=4, name=f"fo_{qc}_{tb}")
                fo = fo_tiles[tb]
                tg, bf = TAIL_TAGS[(4 * tb + jn) % 4]
                psf = psum.tile([128, NQ], F32, tag=tg, bufs=bf,
                                name=f"ps_ft_{qc}_{tb}_{jn}")
                for h in range(HPC):
                    nc.tensor.matmul(
                        psf[:, :],
                        oTs[h][:, ts(tb, 128)],
                        wp_sb[:, h, ds(jn * NQ, NQ)],
                        start=(h == 0), stop=(h == HPC - 1),
                    )
                nc.vector.tensor_copy(fo[:, jn, 0:256], psf[:, 0:256])
                nc.scalar.copy(fo[:, jn, 256:NQ], psf[:, 256:NQ])
                if jn % 2 == 1:
                    # per-half DMA (256KB), alternating rings (both idle
                    # at tail) so the final piece drains early
                    half = jn // 2
                    eng = nc.sync if (2 * tb + half) % 2 == 0 else nc.scalar
                    eng.dma_start(
                        out[ds(qc * NQ + tb * 128, 128),
                            ds(half * 2 * NQ, 2 * NQ)],
                        fo[:, ds(2 * half, 2), :])

            # ---------------- attention with filler injection --------------
            def attn2(qc, fillers):
                # both heads interleaved: two independent exp chains keep
                # the in-order PE queue fed; each head's denominator
                # accumulates into its own partition row (0 / 64) of one
                # shared psum bank via PE column-group tile_position
                diag = list(range(4 * qc, 4 * qc + 4))
                full = list(range(4 * qc))
                order = diag + full
                if qc == 0:
                    pairs = [(diag[0], diag[1]), (diag[2], diag[3])]
                else:
                    pairs = [(diag[i], full[i]) for i in range(4)]
                    rest = full[4:]
                    pairs += [(rest[i], rest[i + 1])
                              for i in range(0, len(rest), 2)]
                first, last = order[0], order[-1]
                np_ = len(pairs)
                ps_o = [psum.tile([128, NQ], F32, tag="o", bufs=2,
                                  name=f"ps_o_{h}_{qc}") for h in range(2)]
                ps_l = psum.tile([128, NQ], F32, tag="l", bufs=1,
                                 name=f"ps_l_{qc}")

                def offw(sb):
                    r = sb - 4 * qc
                    return (128 * r if r > 0 else 0), r

                # cost-weighted filler schedule: a v group (16 MMs)
                # costs ~4 psf groups (2 MMs each); v_i must land by
                # slot i (pair i's o-matmuls read it)
                fcost = [4.0 if c == "v" else 1.0 for c, _, _ in fillers]
                total_u = sum(fcost)
                slots_f = [[] for _ in range(np_)]
                cum = 0.0
                fi = 0
                for pi in range(np_):
                    target = total_u * (pi + 1) / np_
                    while fi < len(fillers) and (
                            cum < target or
                            (fi < len(fillers) and fillers[fi][0] == "v"
                             and fillers[fi][2] <= pi)):
                        slots_f[pi].append(fillers[fi][1])
                        cum += fcost[fi]
                        fi += 1
                while fi < len(fillers):
                    slots_f[-1].append(fillers[fi][1])
                    fi += 1
                for pi, (a, b) in enumerate(pairs):
                    es = {}
                    slot_fill = list(slots_f[pi])

                    def emit_st(h, sb):
                        qm, km = h, 2 + h
                        off, r = offw(sb)
                        ps_s = psum.tile([128, NQ], F32, tag="mm", bufs=4,
                                         name=f"ps_s_{h}_{qc}_{sb}")
                        nc.tensor.matmul(
                            ps_s[:, off:NQ],
                            qk_sb[:, km, ts(sb, 128)],
                            qk_sb[:, qm, ds(qc * NQ + off, NQ - off)],
                            start=True, stop=True,
                        )
                        e = work.tile([128, NQ], BF16, tag="e", bufs=8,
                                      name=f"e_{h}_{qc}_{sb}")
                        nc.scalar.activation(e[:, off:NQ], ps_s[:, off:NQ],
                                             EXP, scale=SCALE)
                        if r >= 0:
                            nc.vector.tensor_mul(
                                e[:, ds(off, 128)], e[:, ds(off, 128)],
                                mask_sb[:, :])
                        es[(h, sb)] = (e, off)

                    # 4 score matmuls share 3 "mm" psum banks: run a
                    # filler between ST#3 and ST#4 so the first exp can
                    # free its bank; remaining fillers occupy the PE
                    # during the exp+mask latency before the o matmuls
                    emit_st(0, a)
                    emit_st(0, b)
                    emit_st(1, a)
                    if slot_fill:
                        slot_fill.pop(0)()
                    emit_st(1, b)
                    for fn in slot_fill:
                        fn()
                    for h in range(2):
                        for sb in (a, b):
                            e, off = es[(h, sb)]
                            nc.tensor.matmul(
                                ps_o[h][:, off:NQ],
                                v_sb[:, sb, ts(h, D)],
                                e[:, off:NQ],
                                start=(sb == first), stop=(sb == last),
                            )
                    for h in range(2):
                        for sb in (a, b):
                            e, off = es[(h, sb)]
                            nc.tensor.matmul(
                                ps_l[64 * h:64 * h + 1, off:NQ],
                                ones_col[:, :], e[:, off:NQ],
                                start=(sb == first), stop=(sb == last),
                                tile_position=(0, 64 * h),
                            )
                oTs = []
                for h in range(2):
                    lsum = work.tile([1, NQ], F32, tag="lsum", bufs=2,
                                     name=f"lsum_{h}_{qc}")
                    nc.vector.tensor_copy(lsum[:, :],
                                          ps_l[64 * h:64 * h + 1, :])
                    bc = work.tile([128, NQ], F32, tag="bc", bufs=2,
                                   name=f"bc_{h}_{qc}")
                    nc.gpsimd.partition_broadcast(bc[:, :], lsum[:, :])
                    rb = work.tile([128, NQ], F32, tag="rb", bufs=2,
                                   name=f"rb_{h}_{qc}")
                    nc.vector.reciprocal_approx_fast(rb[:, :], bc[:, :])
                    oT = work.tile([128, NQ], BF16, tag="oT", bufs=6,
                                   name=f"oT_{h}_{qc}")
                    nc.vector.tensor_mul(oT[:, :], ps_o[h][:, :], rb[:, :])
                    oTs.append(oT)
                return oTs

            # chunk order 0,1,3,2: the kernel ends on the lighter qc2
            # (its slots filled with cproj(3) groups); qc3 gets v8-15 as
            # deadline-pinned fillers (v12-15 feed its diagonal pairs,
            # v8-11 its last full-block pairs)
            V_PLANS = {0: [(0, 0), (1, 1), (2, 2), (3, 3)],
                       1: [(4, 0), (5, 1), (6, 2), (7, 3)],
                       3: [(12, 0), (13, 1), (14, 2), (15, 3),
                           (8, 6), (9, 6), (10, 7), (11, 7)],
                       2: []}
            pending = None   # (qc, [oT_h0, oT_h1]) one chunk behind
            for qc in (0, 1, 3, 2):
                f0 = [("v", (lambda tb=tb: emit_v(tb)), dl)
                      for tb, dl in V_PLANS[qc]]
                f1 = []
                if pending is not None:
                    pq, poTs = pending
                    f1 = [("p",
                           (lambda tb=tb, jn=jn: emit_psf(pq, poTs, tb, jn)),
                           -1)
                          for tb in range(4) for jn in range(QC)]
                pending = (qc, attn2(qc, f0 + f1))
                if qc == 0:
                    emit_wave2(2)
                elif qc == 1:
                    emit_wave2(3)
            # bridge the final normalization chain (lsum->bcast->recip->
            # mul feeds the tail's first psf) with dummy matmuls so the
            # PE never idles >3.4us and the HAM clock stays at 2.4 GHz
            ps_wt = psum.tile([128, NQ], F32, tag="mm", bufs=4,
                              name="ps_warm_tail")
            for i in range(14):
                nc.tensor.matmul(ps_wt[:, :], warm_sb[:, 0:128],
                                 warm_sb[:, :], start=(i == 0),
                                 stop=(i == 13))
            nc.vector.tensor_copy(scrap[:, :], ps_wt[:, :])
            pq, poTs = pending
            for tb in range(4):
                for jn in range(QC):
                    emit_psf_tail(pq, poTs, tb, jn)

    nc.compile()
    return nc


def make_mask() -> np.ndarray:
    # mask[s, t'] = 1 if t' >= s (key s allowed for query t' within the
    # 128x128 block that straddles the causal diagonal)
    s = np.arange(128)[:, None]
    tp = np.arange(128)[None, :]
    return (tp >= s).astype(ml_dtypes.bfloat16)


def prep_inputs(x, W_attn, W_proj):
    bf = ml_dtypes.bfloat16
    xT_np = np.ascontiguousarray(x.T).astype(bf)                   # (C, T)
    # slot layout [p, (g n), kb_in_g*512+t']: DMA-contiguous per slot
    x_t = np.ascontiguousarray(
        xT_np.reshape(4, 4, 128, 4, 512)       # [g][i][p][n][t']
        .transpose(2, 0, 3, 1, 4)              # [p][g][n][i][t']
        .reshape(128, KB * T))
    mask_np = make_mask()
    Wq, Wk, Wv = W_attn[:C], W_attn[C:2 * C], W_attn[2 * C:]
    WpT = W_proj.T  # (C_in, C_out): [i, j]

    def tile_p(a):
        # (KB*128, m) -> [p, kb*m] matching the SBUF [128, kb, m] layout
        kb, m = a.shape[0] // 128, a.shape[1]
        return np.ascontiguousarray(
            a.reshape(kb, 128, m).transpose(1, 0, 2).reshape(128, kb * m))

    in_maps = []
    for c in range(NC):
        sl = slice(c * DH, (c + 1) * DH)
        wqk_c = np.concatenate([Wq[sl], Wk[sl]], axis=0)          # (512, C)
        wqkT_c = np.ascontiguousarray(wqk_c.T).astype(bf)          # (C, 512)
        wvT_c = np.ascontiguousarray(Wv[sl].T).astype(bf)          # (C, 256)
        wpT_c = np.ascontiguousarray(WpT[sl, :]).astype(bf)        # (256, C)
        in_maps.append({
            "xT": x_t, "wqkT": tile_p(wqkT_c), "wvT": tile_p(wvT_c),
            "wpT": tile_p(wpT_c), "maskT": mask_np,
        })
    return in_maps


def assemble(results) -> np.ndarray:
    acc = results[0]["out"].astype(np.float32)
    for c in range(1, NC):
        acc = acc + results[c]["out"].astype(np.float32)
    return acc


def kernel(x: np.ndarray, W_attn: np.ndarray, W_proj: np.ndarray) -> np.ndarray:
    x = np.asarray(x, dtype=np.float32)
    W_attn = np.asarray(W_attn, dtype=np.float32)
    W_proj = np.asarray(W_proj, dtype=np.float32)
    if "nc" not in _CACHED:
        _CACHED["nc"] = build()
    nc = _CACHED["nc"]
    in_maps = prep_inputs(x, W_attn, W_proj)
    try:
        res = run_bass_kernel_spmd(nc, in_maps, core_ids=list(range(NC)))
    except Exception:
        # rare transient device-unrecoverable states heal on retry
        res = run_bass_kernel_spmd(nc, in_maps, core_ids=list(range(NC)))
    return assemble(res.results)


# revision 48
# speedup vs baseline: 1.0499x; 1.0499x over previous
"""Causal self-attention (T=2048, C=2048, 16 heads) on 8 TRN2 NeuronCores.

Tensor-parallel over heads: core c owns heads 2c, 2c+1; no collectives —
each core computes the PARTIAL final projection over its 2 head sections
and the host sums the 8 fp16 partials in fp32 (the "all-reduce after
c_proj" happens at host-gather time).

Per-core kernel:
 - q,k projected transposed (qT,kT: [d, T]) in two query-half waves; v in
   natural layout [T, d], emitted as filler inside attention (below)
 - scores transposed: ST[s, t] = kT_blk.T @ qT (keys on partitions),
   causally trimmed at 128-column granularity
 - softmax without max-subtraction (scores are O(+-6) for randn inputs):
   E = exp(scale*ST) masked only on the 128-col diagonal-straddling
   block; denominator l via ones-matmuls, both heads sharing one psum
   bank in PE column-groups 0/64 (tile_position); o = (v.T @ E)*bc(1/l)
 - both heads run interleaved per query chunk: two independent exp
   chains + v/c_proj filler matmuls (cost-weighted, deadline-pinned)
   keep the strictly in-order PE queue from head-of-line blocking on
   the scalar engine's exp latency
 - throughput details: host pre-tiles all inputs into exact SBUF layout
   (fully contiguous DMAs), single sync HWDGE ring in consumption order
   (the 2 rings share HBM bandwidth and completion-semaphore lanes),
   dummy matmuls warm the HAM clock gate (1.2->2.4 GHz) during the DMA
   leading edge and bridge the final normalization chain before the
   eagerly-emitted last c_proj chunk.
"""

import numpy as np
import ml_dtypes

import concourse.mybir as mybir
import concourse.tile as tile
from concourse import bacc
from concourse.bass import ds, ts
from concourse.bass_utils import run_bass_kernel_spmd

T = 2048
C = 2048
H = 16
D = 128            # head dim
NC = 8             # cores
HPC = H // NC      # heads per core
DH = HPC * D       # 256: qkv rows per section per core
KB = C // 128      # 16 contraction tiles
TB = T // 128      # 16 t tiles
NQ = 512           # query chunk (psum bank width)
QC = T // NQ       # 4 query chunks
SCALE = float(1.0 / np.sqrt(D))

BF16 = mybir.dt.bfloat16
F16 = mybir.dt.float16
F32 = mybir.dt.float32
EXP = mybir.ActivationFunctionType.Exp

_CACHED = {}


def build():
    nc = bacc.Bacc("TRN2", target_bir_lowering=False, debug=False,
                   num_devices=NC)
    # all inputs are pre-tiled on the host into the exact SBUF layout so
    # every DMA is fully contiguous on both sides (large descriptor
    # runs -> cheap triggers, full HBM bandwidth).
    # xT: [p, slot=(g*4+n), kb_in_g*512 + t'] with kb = 4g+kb_in_g,
    # t = 512n + t'
    xT = nc.dram_tensor("xT", [128, KB * T], BF16, kind="ExternalInput")
    wqkT = nc.dram_tensor("wqkT", [128, KB * 4 * D], BF16,
                          kind="ExternalInput")
    wvT = nc.dram_tensor("wvT", [128, KB * DH], BF16, kind="ExternalInput")
    wpT = nc.dram_tensor("wpT", [128, HPC * C], BF16, kind="ExternalInput")
    maskT = nc.dram_tensor("maskT", [128, 128], BF16, kind="ExternalInput")
    out = nc.dram_tensor("out", [T, C], F16, kind="ExternalOutput")

    with tile.TileContext(nc) as tc:
        with (
            tc.tile_pool(name="const", bufs=1) as const,
            tc.tile_pool(name="work", bufs=4) as work,
            tc.tile_pool(name="psum", bufs=2, space="PSUM") as psum,
        ):
            # ---------------- input loads ----------------
            # xT_sb: [p, slot=(g*4+n), kb_in_g*512 + t']
            xT_sb = const.tile([128, KB, T], BF16)
            wqk_sb = const.tile([128, KB, 4 * D], BF16)
            wv_sb = const.tile([128, KB, DH], BF16)
            wp_sb = const.tile([128, HPC, C], BF16)
            mask_sb = const.tile([128, 128], BF16)
            warm_sb = const.tile([128, NQ], BF16)
            wqk_r = wqkT.ap().rearrange("p (kb m) -> p kb m", m=4 * D)
            xT_r = xT.ap().rearrange("p (s w) -> p s w", w=T)

            def xs(kb, t0, w):
                # x[kb, t0:t0+w] in slot layout (t0 must stay within one
                # 512-column n-chunk)
                g, i = kb // 4, kb % 4
                n, tp = t0 // NQ, t0 % NQ
                return xT_sb[:, 4 * g + n, ds(512 * i + tp, w)]

            # PE warm-up: the HAM clock gate needs ~3.4us of sustained
            # matmul activity to lift the PE from 1.2 to 2.4 GHz; burn it
            # on dummy matmuls while the first input DMAs are in flight
            nc.vector.memset(warm_sb[:, :], 0.03125)
            ps_w = psum.tile([128, NQ], F32, tag="f", bufs=2, name="ps_warm")
            for i in range(8):
                nc.tensor.matmul(ps_w[:, :], warm_sb[:, 0:128], warm_sb[:, :],
                                 start=(i == 0), stop=(i == 7))
            scrap = work.tile([128, NQ], F32, tag="scrap", bufs=1,
                              name="scrap")
            nc.vector.tensor_copy(scrap[:, :], ps_w[:, :])

            # ALL loads go on the single sync ring in exact consumption
            # order: the two HWDGE rings share the ~358 GB/s HBM port AND
            # the 8 global DMA-completion semaphore lanes, so a second
            # ring only creates cross-ring stalls. Wave 1 runs n-major
            # (all m at n=0 first), so slots arrive just in time.
            # ultra-fine leading edge: the first wave matmul needs only
            # wqk[kb0] (131KB) + x slot0 cols 0:512 (131KB); wave 1's
            # first kb-group runs kb-major to match this arrival order
            nc.sync.dma_start(wqk_sb[:, 0, :], wqk_r[:, 0, :])
            nc.sync.dma_start(xT_sb[:, 0, ts(0, NQ)], xT_r[:, 0, ts(0, NQ)])
            nc.sync.dma_start(wqk_sb[:, 1, :], wqk_r[:, 1, :])
            nc.sync.dma_start(xT_sb[:, 0, ts(1, NQ)], xT_r[:, 0, ts(1, NQ)])
            nc.sync.dma_start(wqk_sb[:, ds(2, 2), :].rearrange("p a b -> p (a b)"),
                              wqkT.ap()[:, ds(2 * 4 * D, 2 * 4 * D)])
            nc.sync.dma_start(xT_sb[:, 0, ds(2 * NQ, T - 2 * NQ)],
                              xT_r[:, 0, ds(2 * NQ, T - 2 * NQ)])
            nc.sync.dma_start(xT_sb[:, 1, :], xT_r[:, 1, :])
            for g in range(1, 4):
                nc.sync.dma_start(wqk_sb[:, ts(g, 4), :].rearrange("p a b -> p (a b)"),
                                  wqkT.ap()[:, ds(g * 4 * 4 * D, 4 * 4 * D)])
                nc.sync.dma_start(xT_sb[:, 4 * g, :], xT_r[:, 4 * g, :])
                nc.sync.dma_start(xT_sb[:, 4 * g + 1, :], xT_r[:, 4 * g + 1, :])
            nc.sync.dma_start(mask_sb[:, :], maskT[:, :])
            nc.sync.dma_start(wv_sb[:, :, :].rearrange("p a b -> p (a b)"),
                              wvT.ap())
            for g in range(4):
                for n in range(2, 4):
                    nc.sync.dma_start(xT_sb[:, 4 * g + n, :],
                                      xT_r[:, 4 * g + n, :])
            nc.sync.dma_start(wp_sb[:, :, :].rearrange("p a b -> p (a b)"),
                              wpT.ap())

            ones_col = const.tile([128, 1], BF16)
            nc.vector.memset(ones_col[:, :], 1.0)

            qk_sb = const.tile([128, 4, T], BF16)      # m: qh0 qh1 kh0 kh1
            v_sb = const.tile([128, TB, DH], BF16)     # v[tb] natural layout

            # ------------- q,k projections, wave 1 (n = 0, 1) -------------
            wave = [(m, n) for n in range(2) for m in range(4)]
            wtags = [("mm", 3)] * 3 + [("o", 2)] * 2 + [("l", 1)] + \
                [("f", 2)] * 2
            wave_ps = {}
            for (m, n), (tg, bf) in zip(wave, wtags):
                wave_ps[(m, n)] = psum.tile([128, NQ], F32, tag=tg, bufs=bf,
                                            name=f"ps_qk_{m}_{n}")
            # first kb-group kb-major (matches the fine leading DMAs);
            # later groups (m,n)-major with kb runs
            for n in range(2):
                for kb in range(4):
                    for m in range(4):
                        nc.tensor.matmul(
                            wave_ps[(m, n)][:, :],
                            wqk_sb[:, kb, ts(m, 128)],
                            xs(kb, n * NQ, NQ),
                            start=(kb == 0), stop=False,
                        )
            for kbg in range(1, 4):
                for m, n in wave:
                    for kb in range(4 * kbg, 4 * kbg + 4):
                        nc.tensor.matmul(
                            wave_ps[(m, n)][:, :],
                            wqk_sb[:, kb, ts(m, 128)],
                            xs(kb, n * NQ, NQ),
                            start=False, stop=(kb == KB - 1),
                        )
            for m, n in wave:
                nc.vector.tensor_copy(qk_sb[:, m, ts(n, NQ)],
                                      wave_ps[(m, n)][:, :])

            # ------------- q,k projections, wave 2 (n = 2, 3) -------------
            def emit_wave2(n):
                for m in range(4):
                    ps = psum.tile([128, NQ], F32, tag="mm", bufs=3,
                                   name=f"ps_qk_{m}_{n}")
                    for kb in range(KB):
                        nc.tensor.matmul(
                            ps[:, :],
                            wqk_sb[:, kb, ts(m, 128)],
                            xs(kb, n * NQ, NQ),
                            start=(kb == 0), stop=(kb == KB - 1),
                        )
                    nc.vector.tensor_copy(qk_sb[:, m, ts(n, NQ)], ps[:, :])

            # ---------------- filler emitters ----------------
            def emit_v(tb):
                # v[tb] natural layout; psum shares the "f" tag (cproj
                # fillers and v fillers never overlap within a chunk)
                psv = psum.tile([128, DH], F32, tag="f", bufs=2,
                                name=f"ps_v_{tb}")
                for kb in range(KB):
                    nc.tensor.matmul(
                        psv[:, :],
                        xs(kb, tb * 128, 128),
                        wv_sb[:, kb, :],
                        start=(kb == 0), stop=(kb == KB - 1),
                    )
                nc.vector.tensor_copy(v_sb[:, tb, :], psv[:, :])

            fo_tiles = {}

            def emit_psf(qc, oTs, tb, jn):
                # one c_proj psum group: partial[t, j] for a 128x512 block
                if jn == 0:
                    fo_tiles[tb] = work.tile([128, QC, NQ], F16, tag="fo",
                                             bufs=4, name=f"fo_{qc}_{tb}")
                fo = fo_tiles[tb]
                psf = psum.tile([128, NQ], F32, tag="f", bufs=2,
                                name=f"ps_f_{qc}_{tb}_{jn}")
                for h in range(HPC):
                    nc.tensor.matmul(
                        psf[:, :],
                        oTs[h][:, ts(tb, 128)],
                        wp_sb[:, h, ds(jn * NQ, NQ)],
                        start=(h == 0), stop=(h == HPC - 1),
                    )
                if jn == 1:
                    nc.scalar.copy(fo[:, jn, :], psf[:, :])
                else:
                    nc.vector.tensor_copy(fo[:, jn, :], psf[:, :])
                if jn == QC - 1:
                    nc.sync.dma_start(out[ds(qc * NQ + tb * 128, 128), :],
                                      fo[:, :, :])

            TAIL_TAGS = [("f", 2), ("o", 2), ("mm", 3), ("l", 1)]

            def emit_psf_tail(qc, oTs, tb, jn):
                # tail variant: no attention matmuls left to hide the
                # psum->sbuf copy latency, so rotate psf groups across
                # ALL psum tags (every bank is free here), split each
                # copy across DVE and ACT, and DMA out per-psf
                if jn == 0:
                    fo_tiles[tb] = work.tile([128, QC, NQ], F16, tag="fo",
                                             bufs=4, name=f"fo_{qc}_{tb}")
                fo = fo_tiles[tb]
                tg, bf = TAIL_TAGS[(4 * tb + jn) % 4]
                psf = psum.tile([128, NQ], F32, tag=tg, bufs=bf,
                                name=f"ps_ft_{qc}_{tb}_{jn}")
                for h in range(HPC):
                    nc.tensor.matmul(
                        psf[:, :],
                        oTs[h][:, ts(tb, 128)],
                        wp_sb[:, h, ds(jn * NQ, NQ)],
                        start=(h == 0), stop=(h == HPC - 1),
                    )
                nc.vector.tensor_copy(fo[:, jn, 0:256], psf[:, 0:256])
                nc.scalar.copy(fo[:, jn, 256:NQ], psf[:, 256:NQ])
                if jn % 2 == 1:
                    # per-half DMA (256KB), alternating rings (both idle
                    # at tail) so the final piece drains early
                    half = jn // 2
                    eng = nc.sync if (2 * tb + half) % 2 == 0 else nc.scalar
                    eng.dma_start(
                        out[ds(qc * NQ + tb * 128, 128),
                            ds(half * 2 * NQ, 2 * NQ)],
                        fo[:, ds(2 * half, 2), :])

            # ---------------- attention with filler injection --------------
            def attn2(qc, fillers):
                # both heads interleaved: two independent exp chains keep
                # the in-order PE queue fed; each head's denominator
                # accumulates into its own partition row (0 / 64) of one
                # shared psum bank via PE column-group tile_position
                diag = list(range(4 * qc, 4 * qc + 4))
                full = list(range(4 * qc))
                order = diag + full
                if qc == 0:
                    pairs = [(diag[0], diag[1]), (diag[2], diag[3])]
                else:
                    pairs = [(diag[i], full[i]) for i in range(4)]
                    rest = full[4:]
                    pairs += [(rest[i], rest[i + 1])
                              for i in range(0, len(rest), 2)]
                first, last = order[0], order[-1]
                np_ = len(pairs)
                ps_o = [psum.tile([128, NQ], F32, tag="o", bufs=2,
                                  name=f"ps_o_{h}_{qc}") for h in range(2)]
                ps_l = psum.tile([128, NQ], F32, tag="l", bufs=1,
                                 name=f"ps_l_{qc}")

                def offw(sb):
                    r = sb - 4 * qc
                    return (128 * r if r > 0 else 0), r

                # cost-weighted filler schedule: a v group (16 MMs)
                # costs ~4 psf groups (2 MMs each); v_i must land by
                # slot i (pair i's o-matmuls read it)
                fcost = [4.0 if c == "v" else 1.0 for c, _, _ in fillers]
                total_u = sum(fcost)
                slots_f = [[] for _ in range(np_)]
                cum = 0.0
                fi = 0
                for pi in range(np_):
                    target = total_u * (pi + 1) / np_
                    while fi < len(fillers) and (
                            cum < target or
                            (fi < len(fillers) and fillers[fi][0] == "v"
                             and fillers[fi][2] <= pi)):
                        slots_f[pi].append(fillers[fi][1])
                        cum += fcost[fi]
                        fi += 1
                while fi < len(fillers):
                    slots_f[-1].append(fillers[fi][1])
                    fi += 1
                for pi, (a, b) in enumerate(pairs):
                    es = {}
                    slot_fill = list(slots_f[pi])

                    def emit_st(h, sb):
                        qm, km = h, 2 + h
                        off, r = offw(sb)
                        ps_s = psum.tile([128, NQ], F32, tag="mm", bufs=3,
                                         name=f"ps_s_{h}_{qc}_{sb}")
                        nc.tensor.matmul(
                            ps_s[:, off:NQ],
                            qk_sb[:, km, ts(sb, 128)],
                            qk_sb[:, qm, ds(qc * NQ + off, NQ - off)],
                            start=True, stop=True,
                        )
                        e = work.tile([128, NQ], BF16, tag="e", bufs=12,
                                      name=f"e_{h}_{qc}_{sb}")
                        nc.scalar.activation(e[:, off:NQ], ps_s[:, off:NQ],
                                             EXP, scale=SCALE)
                        if r >= 0:
                            nc.vector.tensor_mul(
                                e[:, ds(off, 128)], e[:, ds(off, 128)],
                                mask_sb[:, :])
                        es[(h, sb)] = (e, off)

                    # 4 score matmuls share 3 "mm" psum banks: run a
                    # filler between ST#3 and ST#4 so the first exp can
                    # free its bank; remaining fillers occupy the PE
                    # during the exp+mask latency before the o matmuls
                    emit_st(0, a)
                    emit_st(0, b)
                    emit_st(1, a)
                    if slot_fill:
                        slot_fill.pop(0)()
                    emit_st(1, b)
                    for fn in slot_fill:
                        fn()
                    for h in range(2):
                        for sb in (a, b):
                            e, off = es[(h, sb)]
                            nc.tensor.matmul(
                                ps_o[h][:, off:NQ],
                                v_sb[:, sb, ts(h, D)],
                                e[:, off:NQ],
                                start=(sb == first), stop=(sb == last),
                            )
                    for h in range(2):
                        for sb in (a, b):
                            e, off = es[(h, sb)]
                            nc.tensor.matmul(
                                ps_l[64 * h:64 * h + 1, off:NQ],
                                ones_col[:, :], e[:, off:NQ],
                                start=(sb == first), stop=(sb == last),
                                tile_position=(0, 64 * h),
                            )
                oTs = []
                for h in range(2):
                    lsum = work.tile([1, NQ], F32, tag="lsum", bufs=3,
                                     name=f"lsum_{h}_{qc}")
                    nc.vector.tensor_copy(lsum[:, :],
                                          ps_l[64 * h:64 * h + 1, :])
                    bc = work.tile([128, NQ], F32, tag="bc", bufs=3,
                                   name=f"bc_{h}_{qc}")
                    nc.gpsimd.partition_broadcast(bc[:, :], lsum[:, :])
                    rb = work.tile([128, NQ], F32, tag="rb", bufs=3,
                                   name=f"rb_{h}_{qc}")
                    nc.vector.reciprocal_approx_fast(rb[:, :], bc[:, :])
                    oT = work.tile([128, NQ], BF16, tag="oT", bufs=8,
                                   name=f"oT_{h}_{qc}")
                    nc.vector.tensor_mul(oT[:, :], ps_o[h][:, :], rb[:, :])
                    oTs.append(oT)
                return oTs

            # chunk order 0,1,3,2: the kernel ends on the lighter qc2
            # (its slots filled with cproj(3) groups); qc3 gets v8-15 as
            # deadline-pinned fillers (v12-15 feed its diagonal pairs,
            # v8-11 its last full-block pairs)
            V_PLANS = {0: [(0, 0), (1, 1), (2, 2), (3, 3)],
                       1: [(4, 0), (5, 1), (6, 2), (7, 3)],
                       3: [(12, 0), (13, 1), (14, 2), (15, 3),
                           (8, 6), (9, 6), (10, 7), (11, 7)],
                       2: []}
            pending = None   # (qc, [oT_h0, oT_h1]) one chunk behind
            for qc in (0, 1, 3, 2):
                f0 = [("v", (lambda tb=tb: emit_v(tb)), dl)
                      for tb, dl in V_PLANS[qc]]
                f1 = []
                if pending is not None:
                    pq, poTs = pending
                    f1 = [("p",
                           (lambda tb=tb, jn=jn: emit_psf(pq, poTs, tb, jn)),
                           -1)
                          for tb in range(4) for jn in range(QC)]
                pending = (qc, attn2(qc, f0 + f1))
                if qc == 0:
                    emit_wave2(2)
                elif qc == 1:
                    emit_wave2(3)
            # bridge the final normalization chain (lsum->bcast->recip->
            # mul feeds the tail's first psf) with dummy matmuls so the
            # PE never idles >3.4us and the HAM clock stays at 2.4 GHz
            ps_wt = psum.tile([128, NQ], F32, tag="mm", bufs=3,
                              name="ps_warm_tail")
            for i in range(14):
                nc.tensor.matmul(ps_wt[:, :], warm_sb[:, 0:128],
                                 warm_sb[:, :], start=(i == 0),
                                 stop=(i == 13))
            nc.vector.tensor_copy(scrap[:, :], ps_wt[:, :])
            pq, poTs = pending
            for tb in range(4):
                for jn in range(QC):
                    emit_psf_tail(pq, poTs, tb, jn)

    nc.compile()
    return nc


def make_mask() -> np.ndarray:
    # mask[s, t'] = 1 if t' >= s (key s allowed for query t' within the
    # 128x128 block that straddles the causal diagonal)
    s = np.arange(128)[:, None]
    tp = np.arange(128)[None, :]
    return (tp >= s).astype(ml_dtypes.bfloat16)


def prep_inputs(x, W_attn, W_proj):
    bf = ml_dtypes.bfloat16
    xT_np = np.ascontiguousarray(x.T).astype(bf)                   # (C, T)
    # slot layout [p, (g n), kb_in_g*512+t']: DMA-contiguous per slot
    x_t = np.ascontiguousarray(
        xT_np.reshape(4, 4, 128, 4, 512)       # [g][i][p][n][t']
        .transpose(2, 0, 3, 1, 4)              # [p][g][n][i][t']
        .reshape(128, KB * T))
    mask_np = make_mask()
    Wq, Wk, Wv = W_attn[:C], W_attn[C:2 * C], W_attn[2 * C:]
    WpT = W_proj.T  # (C_in, C_out): [i, j]

    def tile_p(a):
        # (KB*128, m) -> [p, kb*m] matching the SBUF [128, kb, m] layout
        kb, m = a.shape[0] // 128, a.shape[1]
        return np.ascontiguousarray(
            a.reshape(kb, 128, m).transpose(1, 0, 2).reshape(128, kb * m))

    in_maps = []
    for c in range(NC):
        sl = slice(c * DH, (c + 1) * DH)
        wqk_c = np.concatenate([Wq[sl], Wk[sl]], axis=0)          # (512, C)
        wqkT_c = np.ascontiguousarray(wqk_c.T).astype(bf)          # (C, 512)
        wvT_c = np.ascontiguousarray(Wv[sl].T).astype(bf)          # (C, 256)
        wpT_c = np.ascontiguousarray(WpT[sl, :]).astype(bf)        # (256, C)
        in_maps.append({
            "xT": x_t, "wqkT": tile_p(wqkT_c), "wvT": tile_p(wvT_c),
            "wpT": tile_p(wpT_c), "maskT": mask_np,
        })
    return in_maps


def assemble(results) -> np.ndarray:
    acc = results[0]["out"].astype(np.float32)
    for c in range(1, NC):
        acc = acc + results[c]["out"].astype(np.float32)
    return acc


def kernel(x: np.ndarray, W_attn: np.ndarray, W_proj: np.ndarray) -> np.ndarray:
    x = np.asarray(x, dtype=np.float32)
    W_attn = np.asarray(W_attn, dtype=np.float32)
    W_proj = np.asarray(W_proj, dtype=np.float32)
    if "nc" not in _CACHED:
        _CACHED["nc"] = build()
    nc = _CACHED["nc"]
    in_maps = prep_inputs(x, W_attn, W_proj)
    try:
        res = run_bass_kernel_spmd(nc, in_maps, core_ids=list(range(NC)))
    except Exception:
        # rare transient device-unrecoverable states heal on retry
        res = run_bass_kernel_spmd(nc, in_maps, core_ids=list(range(NC)))
    return assemble(res.results)
